# revision 10
# baseline (speedup 1.0000x reference)
"""DigitCaps routing kernel for Trainium2 (8 NeuronCores, SPMD data-parallel over batch).

Math (per batch element b):
  u_hat[r, c, o] = sum_i W[r, c, o, i] * x[r, i]
  b_log = 0
  repeat 3x:
    c = softmax(b_log, axis=c)
    s[c, o] = sum_r c[r, c] * u_hat[r, c, o]
    v = squash(s)                               (over o)
    b_log += sum_o u_hat[r, c, o] * v[c, o]     (first 2 iters only)
  return v

Layout strategy per core (B_local = 32 batches):
  - r split into 72 supergroups (rg) of 16 (r16); phase A contracts K =
    (r16, i) = 128 using a host-built block-diagonal x as the stationary
    operand: lhsT[(r16, i), (r16', b8)] = x[b, r, i] * delta(r16, r16').
  - u_hat lives in SBUF as bf16 [(r16, b8)=128 partitions, (bg=4, rg=72, c=11, o=16)].
  - s-step: y = c_sm (.) u_hat (free-dim broadcast over o) on DVE, then the
    r16-partition sum via a constant block-diagonal-ones stationary matmul.
  - agreement: z = u_hat (.) v_t, then o-group tensor_reduce on DVE; v_t is v
    replicated across the 16 r16 partition groups via a tiny PE matmul with a
    constant 0/1 replication matrix.
"""

import sys
for p in ("/opt/trn_rl_repo", "/root/.axon_site/_ro/trn_rl_repo"):
    if p not in sys.path:
        sys.path.insert(0, p)

import numpy as np
import ml_dtypes
from contextlib import ExitStack

import concourse.bass as bass
import concourse.tile as tile
from concourse import bacc, mybir
from concourse.bass_utils import run_bass_kernel_spmd

# problem constants
B, R, C, I, O = 256, 1152, 11, 8, 16
ITERS = 3
EPS = 1e-9

N_CORES = 8
BL = B // N_CORES          # 32 batches per core
RG = R // 16               # 72 supergroups of 16 r
CO = C * O                 # 176
NBG = BL // 8              # 4 b-groups of 8
NT = RG * NBG              # 288 phase-A tiles
STG = 8                    # phase-A tiles per staging DMA
CH = 8                     # rg chunk for premultiplies
F32 = mybir.dt.float32
BF16 = mybir.dt.bfloat16
BF16_NP = ml_dtypes.bfloat16


def _build_program():
    nc = bacc.Bacc("TRN2", target_bir_lowering=False, debug=False, num_devices=N_CORES)

    xbd_d = nc.dram_tensor("xbd", [NT // STG, 128, STG * 128], BF16, kind="ExternalInput").ap()
    wt_d = nc.dram_tensor("wt", [128, RG * CO], BF16, kind="ExternalInput").ap()
    xc_d = nc.dram_tensor("xc", [128, RG * BL], BF16, kind="ExternalInput").ap()
    ones_d = nc.dram_tensor("onesbd", [NBG, 128, 32], BF16, kind="ExternalInput").ap()
    rep_d = nc.dram_tensor("rep", [NBG, 32, 128], F32, kind="ExternalInput").ap()
    vout_d = nc.dram_tensor("vout", [BL, CO], F32, kind="ExternalOutput").ap()

    with tile.TileContext(nc) as tc, ExitStack() as ctx:
        const_p = ctx.enter_context(tc.tile_pool(name="const", bufs=1))
        stg_p = ctx.enter_context(tc.tile_pool(name="stg", bufs=3))
        uh_p = ctx.enter_context(tc.tile_pool(name="uh", bufs=1))
        log_p = ctx.enter_context(tc.tile_pool(name="logit", bufs=1))
        sm_p = ctx.enter_context(tc.tile_pool(name="smx", bufs=1))
        y_p = ctx.enter_context(tc.tile_pool(name="y", bufs=4))
        a_p = ctx.enter_context(tc.tile_pool(name="a", bufs=4))
        vt_p = ctx.enter_context(tc.tile_pool(name="vt", bufs=2))
        sq_p = ctx.enter_context(tc.tile_pool(name="sq", bufs=1))
        psA = ctx.enter_context(tc.tile_pool(name="psA", bufs=4, space=bass.MemorySpace.PSUM))
        psS = ctx.enter_context(tc.tile_pool(name="psS", bufs=2, space=bass.MemorySpace.PSUM))

        w_sb = const_p.tile([128, RG * CO], BF16)
        nc.sync.dma_start(w_sb[:], wt_d[:])
        xc_sb = const_p.tile([128, RG * BL], BF16)
        nc.sync.dma_start(xc_sb[:], xc_d[:])
        ones_sb = const_p.tile([128, NBG * 32], BF16)
        nc.sync.dma_start(
            ones_sb[:].rearrange("p (g m) -> p g m", g=NBG),
            ones_d[:].transpose([1, 0, 2]),
        )
        ones3 = ones_sb[:].rearrange("p (g m) -> p g m", g=NBG)
        rep_sb = const_p.tile([32, NBG * 128], F32)
        nc.sync.dma_start(
            rep_sb[:].rearrange("p (g m) -> p g m", g=NBG),
            rep_d[:].transpose([1, 0, 2]),
        )

        # u_hat free layout: (bg, rg, c, o)
        u_hat = uh_p.tile([128, NBG * RG * CO], BF16)
        uh4 = u_hat[:].rearrange("p (g r f) -> p g r f", g=NBG, r=RG)

        # ---------------- phase A: u_hat ----------------
        for t in range(NT // STG):
            stg = stg_p.tile([128, STG * 128], BF16)
            nc.sync.dma_start(stg[:], xbd_d[t])
            for k in range(STG):
                tidx = t * STG + k
                rg, bg = tidx // NBG, tidx % NBG
                ps = psA.tile([128, CO], F32, tag="ps")
                nc.tensor.matmul(
                    ps[:],
                    stg[:, k * 128:(k + 1) * 128],
                    w_sb[:, rg * CO:(rg + 1) * CO],
                    start=True, stop=True,
                )
                dst = uh4[:, bg, rg, :]
                if tidx % 2 == 0:
                    nc.vector.tensor_copy(dst, ps[:])
                else:
                    nc.scalar.activation(dst, ps[:], mybir.ActivationFunctionType.Copy)

        # logits & softmax buffers (f32)
        b_log = log_p.tile([128, NBG * RG * C], F32)
        bl3 = b_log[:].rearrange("p (g r c) -> p g r c", g=NBG, r=RG)
        exp_t = sm_p.tile([128, NBG * RG * C], F32)
        rsum = sm_p.tile([128, NBG * RG], F32)
        rrec = sm_p.tile([128, NBG * RG], F32)
        c_sm = sm_p.tile([128, NBG * RG * C], BF16)
        cs3 = c_sm[:].rearrange("p (g r c) -> p g r c", g=NBG, r=RG)

        s_sb = sq_p.tile([32, CO], F32)
        sqv = sq_p.tile([32, CO], F32)
        ss = sq_p.tile([32, C], F32)
        t2 = sq_p.tile([32, C], F32)
        sqr = sq_p.tile([32, C], F32)
        den = sq_p.tile([32, C], F32)
        rf = sq_p.tile([32, C], F32)
        fac = sq_p.tile([32, C], F32)
        v_sb = sq_p.tile([32, CO], F32)
        eps_t = sq_p.tile([32, 1], F32)
        nc.vector.memset(eps_t[:], EPS)

        def squash(scale):
            # v_sb = squash(s_sb * scale) over o; all tiny [32, *] ops
            nc.vector.tensor_mul(sqv[:], s_sb[:], s_sb[:])
            nc.vector.tensor_reduce(
                ss[:], sqv[:].rearrange("p (c o) -> p c o", c=C),
                axis=mybir.AxisListType.X, op=mybir.AluOpType.add,
            )
            if scale != 1.0:
                # ss holds |s_unscaled|^2; rescale to |scale*s|^2
                nc.vector.tensor_scalar_mul(ss[:], ss[:], scale * scale)
            nc.scalar.activation(sqr[:], ss[:], mybir.ActivationFunctionType.Sqrt, bias=eps_t[:])
            nc.vector.tensor_scalar_add(t2[:], ss[:], 1.0)
            nc.vector.tensor_mul(den[:], t2[:], sqr[:])
            nc.vector.reciprocal(rf[:], den[:])
            nc.vector.tensor_mul(fac[:], ss[:], rf[:])
            if scale != 1.0:
                nc.vector.tensor_scalar_mul(fac[:], fac[:], scale)
            nc.vector.tensor_mul(
                v_sb[:].rearrange("p (c o) -> p c o", c=C),
                s_sb[:].rearrange("p (c o) -> p c o", c=C),
                fac[:].unsqueeze(2).broadcast_to([32, C, O]),
            )

        def replicate_v():
            # vt_bg[(r16, b8), co] = v[bg*8 + b8, co], via PE with the constant
            # 0/1 replication matrix (f32 matmul, K=32).
            vts = []
            for bg in range(NBG):
                ps = psA.tile([128, CO], F32, tag="ps")
                nc.tensor.matmul(
                    ps[:],
                    rep_sb[:].rearrange("p (g m) -> p g m", g=NBG)[:, bg, :],
                    v_sb[:],
                    start=True, stop=True,
                )
                vt = vt_p.tile([128, CO], BF16, tag=f"vt{bg}")
                nc.vector.tensor_copy(vt[:], ps[:])
                vts.append(vt)
            return vts

        def agreement(vts, first):
            # b_log (+)= sum_o u_hat * v_t
            for bg in range(NBG):
                for r0 in range(0, RG, CH):
                    z = y_p.tile([128, CH * CO], BF16, tag="y")
                    nc.vector.tensor_mul(
                        z[:].rearrange("p (r f) -> p r f", r=CH),
                        uh4[:, bg, r0:r0 + CH, :],
                        vts[bg][:].unsqueeze(1).broadcast_to([128, CH, CO]),
                    )
                    dst = bl3[:, bg, r0:r0 + CH, :]
                    zv = z[:].rearrange("p (r c o) -> p r c o", r=CH, c=C)
                    if first:
                        nc.vector.tensor_reduce(
                            dst, zv, axis=mybir.AxisListType.X, op=mybir.AluOpType.add,
                        )
                    else:
                        ac = a_p.tile([128, CH * C], F32, tag="ac")
                        nc.vector.tensor_reduce(
                            ac[:].rearrange("p (r c) -> p r c", r=CH),
                            zv, axis=mybir.AxisListType.X, op=mybir.AluOpType.add,
                        )
                        nc.vector.tensor_add(
                            dst, dst, ac[:].rearrange("p (r c) -> p r c", r=CH)
                        )

        def softmax():
            nc.scalar.activation(exp_t[:], b_log[:], mybir.ActivationFunctionType.Exp)
            nc.vector.tensor_reduce(
                rsum[:], exp_t[:].rearrange("p (g c) -> p g c", c=C),
                axis=mybir.AxisListType.X, op=mybir.AluOpType.add,
            )
            nc.vector.reciprocal(rrec[:], rsum[:])
            nc.vector.tensor_mul(
                c_sm[:].rearrange("p (g c) -> p g c", c=C),
                exp_t[:].rearrange("p (g c) -> p g c", c=C),
                rrec[:].unsqueeze(2).broadcast_to([128, NBG * RG, C]),
            )

        def s_step():
            # s = sum_r c_sm * u_hat via DVE premultiply + block-diag-ones
            # matmuls; all 4 b-groups accumulate into one [32, CO] PSUM tile
            # (each bg's ones matrix hits a disjoint 8-partition block of M).
            ps = psS.tile([32, CO], F32, tag="psS")
            for bg in range(NBG):
                for r0 in range(0, RG, CH):
                    y = y_p.tile([128, CH * CO], BF16, tag="y")
                    nc.vector.tensor_mul(
                        y[:].rearrange("p (r c o) -> p r c o", r=CH, c=C),
                        uh4[:, bg, r0:r0 + CH, :].rearrange("p r (c o) -> p r c o", c=C),
                        cs3[:, bg, r0:r0 + CH, :].unsqueeze(3).broadcast_to([128, CH, C, O]),
                    )
                    for j in range(CH):
                        rg = r0 + j
                        nc.tensor.matmul(
                            ps[:], ones3[:, bg, :], y[:, j * CO:(j + 1) * CO],
                            start=(bg == 0 and rg == 0),
                            stop=(bg == NBG - 1 and rg == RG - 1),
                        )
            nc.scalar.activation(s_sb[:], ps[:], mybir.ActivationFunctionType.Copy)

        # ---------------- iter 0 ----------------
        # c uniform (1/11): s0 = (1/11) sum_r u_hat, via compact-x matmuls
        ps0 = psS.tile([32, CO], F32, tag="psS")
        for rg in range(RG):
            nc.tensor.matmul(
                ps0[:], xc_sb[:, rg * BL:(rg + 1) * BL], w_sb[:, rg * CO:(rg + 1) * CO],
                start=(rg == 0), stop=(rg == RG - 1),
            )
        nc.scalar.activation(s_sb[:], ps0[:], mybir.ActivationFunctionType.Copy)
        squash(1.0 / C)
        vts = replicate_v()
        agreement(vts, first=True)

        # ---------------- iter 1 ----------------
        softmax()
        s_step()
        squash(1.0)
        vts = replicate_v()
        agreement(vts, first=False)

        # ---------------- iter 2 ----------------
        softmax()
        s_step()
        squash(1.0)
        nc.sync.dma_start(vout_d[:], v_sb[:])

    nc.compile()
    return nc


_CACHE = {}


def _get_program():
    if "nc" not in _CACHE:
        _CACHE["nc"] = _build_program()
    return _CACHE["nc"]


def _host_xbd(x_l):
    """Block-diag x, staged for DMA: [NT//STG, 128, STG*128] bf16."""
    xr = x_l.reshape(BL, RG, 16, I)
    xbd = np.zeros((NT, 128, 128), dtype=BF16_NP)
    blk = xbd.reshape(RG, NBG, 128, 128)
    for r16 in range(16):
        t = xr[:, :, r16, :]                                 # [BL, RG, I]
        t = t.transpose(1, 2, 0)                             # [RG, I, BL]
        t = t.reshape(RG, I, NBG, 8).transpose(0, 2, 1, 3)   # [RG, NBG, I, 8]
        blk[:, :, r16 * 8:(r16 + 1) * 8, r16 * 8:(r16 + 1) * 8] = t.astype(BF16_NP)
    return np.ascontiguousarray(
        xbd.reshape(NT // STG, STG, 128, 128).transpose(0, 2, 1, 3)
        .reshape(NT // STG, 128, STG * 128)
    )


def _make_in_maps(x, W):
    x = np.asarray(x, dtype=np.float32)
    W = np.asarray(W, dtype=np.float32)

    wt = np.ascontiguousarray(
        W.reshape(RG, 16, C, O, I).transpose(1, 4, 0, 2, 3).reshape(128, RG * CO)
    ).astype(BF16_NP)
    ones_bd = np.zeros((NBG, 128, 32), dtype=BF16_NP)
    for bg in range(NBG):
        for p in range(128):
            ones_bd[bg, p, bg * 8 + p % 8] = 1.0
    rep = np.zeros((NBG, 32, 128), dtype=np.float32)
    for bg in range(NBG):
        for r16 in range(16):
            for b8 in range(8):
                rep[bg, bg * 8 + b8, r16 * 8 + b8] = 1.0

    in_maps = []
    for core in range(N_CORES):
        x_l = x[core * BL:(core + 1) * BL]
        xc = np.ascontiguousarray(
            x_l.reshape(BL, RG, 16, I).transpose(2, 3, 1, 0).reshape(128, RG * BL)
        ).astype(BF16_NP)
        in_maps.append({
            "xbd": _host_xbd(x_l),
            "wt": wt,
            "xc": xc,
            "onesbd": ones_bd,
            "rep": rep,
        })
    return in_maps


def kernel(x, W):
    in_maps = _make_in_maps(x, W)
    nc = _get_program()
    res = run_bass_kernel_spmd(nc, in_maps, list(range(N_CORES)))
    out = np.concatenate(
        [res.results[i]["vout"].reshape(BL, C, O) for i in range(N_CORES)], axis=0
    )
    return out.astype(np.float32)


if __name__ == "__main__":
    rng = np.random.default_rng(0)
    x = rng.standard_normal((B, R, I), dtype=np.float32)
    W = (rng.standard_normal((R, C, O, I), dtype=np.float32) * 0.01).astype(np.float32)
    v = kernel(x=x, W=W)
    print("out", v.shape, v.dtype, np.abs(v).mean())
